# revision 1
# baseline (speedup 1.0000x reference)
"""Trainium2 Bass kernel for the RNN-T JointNetwork problem.

Computes log_softmax(tanh(cat(enc, pred)) @ W.T + b) over the vocab dim
for logits of shape [B=4, T=200, U=50, V=1024], fp32.

Strategy (data-parallel over the 800 flattened (b,t) rows, 100 per core):
  setup (per core, on device):
    teT  = tanh(encT_slice)          [512, 100]   (ACT)
    tpT  = tanh(predT_slice)         [512, 50]    (ACT)
    enc_p  = teT.T @ WeT             [100, 1024]  (PE, fp32)
    pred_b = tpT.T @ WpT + b         [50, 1024]   (PE, fp32)
  main loop over 40 row-tiles of 128 rows (row r = t*50 + u):
    x    = u_ind[k].T @ pred_b + t_ind[k].T @ enc_p   (PE -> PSUM, f32r;
           one-hot stationary operands do the broadcast-add, exactly)
    e,s  = exp(x), rowsum(e)         (ACT with accum_out)
    lse  = ln(s)                     (ACT)
    out  = x - lse                   (DVE tensor_scalar, PSUM -> SBUF)
    DMA out tile -> DRAM (round-robin over issue queues)
"""

import numpy as np

import concourse.bass as bass
import concourse.bacc as bacc
import concourse.tile as tile
from concourse import mybir
from concourse.bass_utils import run_bass_kernel_spmd

# Problem shapes (hardcoded per contract).
B, T, U, D, V = 4, 200, 50, 512, 1024
N_CORES = 8
BT = B * T                     # 800 flattened (b,t) rows
TPC = BT // N_CORES            # 100 (b,t) rows per core
ROWS = TPC * U                 # 5000 output rows per core
P = 128
NT = (ROWS + P - 1) // P       # 40 row-tiles per core
NV = V // 512                  # fp32 moving-operand free-dim limit is 512
DC = D // P                    # 4 contraction chunks of 128 for D=512

f32 = mybir.dt.float32
f32r = mybir.dt.float32r
bf16 = mybir.dt.bfloat16

# Which issue queues take the 40 output-tile DMAs, round-robin.
OUT_DMA_ENGINES = ("sync", "gpsimd")

TRACE = False
LAST_RESULT = None

_CACHE = {}


def _patch_act_tables():
    """Pin Exp/Ln to the one table set containing both, so the activation
    table-load pass never alternates sets inside the main loop.

    Claiming a set does NOT contain a function is always safe (it can only
    add loads); here it redirects Exp away from sets lacking Ln.
    """
    if getattr(bacc, "_joint_act_patch", False):
        return
    orig = bacc.get_activation_tables

    def patched(arch):
        t = dict(orig(arch))
        keep = "natural_log_exp_and_others"
        drop = {mybir.ActivationFunctionType.Exp, mybir.ActivationFunctionType.Ln}
        for name, fns in t.items():
            if name != keep:
                t[name] = set(fns) - drop
        return t

    bacc.get_activation_tables = patched
    bacc._joint_act_patch = True


def _build_indicators():
    """Per-row-tile one-hot stationary operands, shared by all cores.

    u_ind[u, k, c] = 1 iff row (128k+c) has u(row) == u  (row % 50)
    t_ind[t, k, c] = 1 iff row (128k+c) has t(row) == t  (row // 50)
    Columns for rows >= ROWS (tail of the last tile) are all-zero.
    """
    r = np.arange(NT * P)
    valid = r < ROWS
    u_ind = np.zeros((U, NT * P), dtype=np.float32)
    t_ind = np.zeros((TPC, NT * P), dtype=np.float32)
    u_ind[(r % U)[valid], r[valid]] = 1.0
    t_ind[(r // U)[valid], r[valid]] = 1.0
    return (
        np.ascontiguousarray(u_ind.reshape(U, NT, P)),
        np.ascontiguousarray(t_ind.reshape(TPC, NT, P)),
    )


def _build_program():
    _patch_act_tables()
    nc = bacc.Bacc("TRN2", target_bir_lowering=False, debug=False,
                   num_devices=N_CORES)

    encT = nc.dram_tensor("encT", [D, TPC], f32, kind="ExternalInput")
    predT = nc.dram_tensor("predT", [D, U], f32, kind="ExternalInput")
    # W in bf16: halves the 4MB load and runs setup matmuls at full PE rate;
    # the resulting ~1e-3 relative rounding of the logits is far inside the
    # output tolerance (outputs are O(1..10), fp32 pipeline elsewhere).
    wT = nc.dram_tensor("wT", [2 * D, V], bf16, kind="ExternalInput")
    bias = nc.dram_tensor("bias", [V], f32, kind="ExternalInput")
    u_ind = nc.dram_tensor("u_ind", [U, NT, P], bf16, kind="ExternalInput")
    t_ind = nc.dram_tensor("t_ind", [TPC, NT, P], bf16, kind="ExternalInput")
    out = nc.dram_tensor("out", [ROWS, V], f32, kind="ExternalOutput")

    with tile.TileContext(nc) as tc:
        with (
            tc.tile_pool(name="consts", bufs=1) as consts,
            tc.tile_pool(name="psum", bufs=4, space=bass.MemorySpace.PSUM) as psum,
            tc.tile_pool(name="scratch", bufs=2) as scratch,
            tc.tile_pool(name="outs", bufs=6) as outs,
            tc.tile_pool(name="small", bufs=8) as small,
        ):
            # ---- load constants / inputs (spread across DMA issue queues,
            #      wT chunked so setup matmuls can start before it finishes) ----
            wt_sb = consts.tile([P, 2 * DC, V], bf16)
            wT_r = wT.ap().rearrange("(c p) v -> p c v", p=P)
            for c in range(2 * DC):
                eng = nc.sync if c % 2 == 0 else nc.scalar
                eng.dma_start(out=wt_sb[:, c, :], in_=wT_r[:, c, :])
            encT_sb = consts.tile([P, DC, TPC], f32)
            nc.scalar.dma_start(out=encT_sb[:], in_=encT.ap().rearrange(
                "(c p) t -> p c t", p=P))
            predT_sb = consts.tile([P, DC, U], f32)
            nc.scalar.dma_start(out=predT_sb[:], in_=predT.ap().rearrange(
                "(c p) u -> p c u", p=P))
            # indicators split so the first tiles don't wait on the full 3MB
            KSPLIT = 6
            uind_sb = consts.tile([U, NT, P], bf16)
            nc.gpsimd.dma_start(out=uind_sb[:, :KSPLIT, :],
                                in_=u_ind.ap()[:, :KSPLIT, :])
            nc.gpsimd.dma_start(out=uind_sb[:, KSPLIT:, :],
                                in_=u_ind.ap()[:, KSPLIT:, :])
            tind_sb = consts.tile([TPC, NT, P], bf16)
            nc.gpsimd.dma_start(out=tind_sb[:, :KSPLIT, :],
                                in_=t_ind.ap()[:, :KSPLIT, :])
            nc.gpsimd.dma_start(out=tind_sb[:, KSPLIT:, :],
                                in_=t_ind.ap()[:, KSPLIT:, :])
            b_sb = consts.tile([1, V], f32)
            nc.scalar.dma_start(out=b_sb[:], in_=bias.ap().rearrange(
                "(p v) -> p v", p=1))
            ones_u = consts.tile([1, U], f32)
            nc.vector.memset(ones_u[:], 1.0)

            # ---- tanh of activations (transposed layout: d on partitions),
            #      bf16 out to pair with the bf16 weights in the setup GEMMs ----
            teT = consts.tile([P, DC, TPC], bf16)
            nc.scalar.activation(teT[:], encT_sb[:],
                                 mybir.ActivationFunctionType.Tanh)
            tpT = consts.tile([P, DC, U], bf16)
            nc.scalar.activation(tpT[:], predT_sb[:],
                                 mybir.ActivationFunctionType.Tanh)

            # ---- enc_p[t, v] = sum_d teT[d, t] * We[v, d] ----
            enc_p = consts.tile([TPC, V], bf16)
            enc_ps = psum.tile([TPC, V], f32, tag="x")
            for vc in range(NV):
                sl = slice(vc * 512, (vc + 1) * 512)
                for c in range(DC):
                    nc.tensor.matmul(enc_ps[:, sl], teT[:, c, :],
                                     wt_sb[:, c, sl],
                                     start=(c == 0), stop=(c == DC - 1))
            nc.vector.tensor_copy(enc_p[:], enc_ps[:])

            # ---- pred_b[u, v] = sum_d tpT[d, u] * Wp[v, d] + b[v] ----
            pred_b = consts.tile([U, V], bf16)
            pred_ps = psum.tile([U, V], f32, tag="x")
            for vc in range(NV):
                sl = slice(vc * 512, (vc + 1) * 512)
                for c in range(DC):
                    nc.tensor.matmul(pred_ps[:, sl], tpT[:, c, :],
                                     wt_sb[:, DC + c, sl],
                                     start=(c == 0), stop=False)
                nc.tensor.matmul(pred_ps[:, sl], ones_u[:], b_sb[:, sl],
                                 start=False, stop=True)
            nc.vector.tensor_copy(pred_b[:], pred_ps[:])

            # ---- main loop over row tiles ----
            dma_engines = [getattr(nc, e) for e in OUT_DMA_ENGINES]
            for k in range(NT):
                r0 = k * P
                rows = min(P, ROWS - r0)
                x_ps = psum.tile([P, V], f32, tag="x")
                # f32r: full-rate fp32 streaming on the PE.  The one-hot
                # stationary operand is exact in any precision; only the
                # pass-through of pred_b/enc_p values sees f32r rounding.
                for vc in range(NV):
                    sl = slice(vc * 512, (vc + 1) * 512)
                    nc.tensor.matmul(x_ps[:rows, sl],
                                     uind_sb[:, k, :rows],
                                     pred_b[:, sl],
                                     start=True, stop=False)
                for vc in range(NV):
                    sl = slice(vc * 512, (vc + 1) * 512)
                    nc.tensor.matmul(x_ps[:rows, sl],
                                     tind_sb[:, k, :rows],
                                     enc_p[:, sl],
                                     start=False, stop=True)
                sums = small.tile([P, 1], f32)
                escr = scratch.tile([P, V], f32)
                nc.scalar.activation(escr[:rows], x_ps[:rows],
                                     mybir.ActivationFunctionType.Exp,
                                     accum_out=sums[:rows])
                lse = small.tile([P, 1], f32)
                nc.scalar.activation(lse[:rows], sums[:rows],
                                     mybir.ActivationFunctionType.Ln)
                o = outs.tile([P, V], f32)
                nc.vector.tensor_scalar_sub(o[:rows], x_ps[:rows], lse[:rows])
                eng = dma_engines[k % len(dma_engines)]
                eng.dma_start(out=out.ap()[r0:r0 + rows, :], in_=o[:rows])

    nc.compile()
    return nc


def kernel(enc_out, pred_out, W, b):
    global LAST_RESULT
    enc_out = np.asarray(enc_out, dtype=np.float32)
    pred_out = np.asarray(pred_out, dtype=np.float32)
    W = np.asarray(W, dtype=np.float32)
    b = np.asarray(b, dtype=np.float32)

    if "nc" not in _CACHE:
        _CACHE["nc"] = _build_program()
        _CACHE["ind"] = _build_indicators()
    nc = _CACHE["nc"]
    u_ind, t_ind = _CACHE["ind"]

    import ml_dtypes
    wT = np.ascontiguousarray(W.T).astype(ml_dtypes.bfloat16)   # [2D, V]
    enc_flat = enc_out.reshape(BT, D)                 # [800, 512]

    in_maps = []
    for c in range(N_CORES):
        bt0 = c * TPC
        b_idx = bt0 // T
        in_maps.append({
            "encT": np.ascontiguousarray(enc_flat[bt0:bt0 + TPC].T),
            "predT": np.ascontiguousarray(pred_out[b_idx].T),
            "wT": wT,
            "bias": b,
            "u_ind": u_ind.astype(ml_dtypes.bfloat16),
            "t_ind": t_ind.astype(ml_dtypes.bfloat16),
        })

    res = run_bass_kernel_spmd(nc, in_maps, core_ids=list(range(N_CORES)),
                               trace=TRACE)
    LAST_RESULT = res
    full = np.concatenate([r["out"] for r in res.results], axis=0)
    return full.reshape(B, T, U, V)



# revision 4
# speedup vs baseline: 1.4073x; 1.4073x over previous
"""Trainium2 Bass kernel for the RNN-T JointNetwork problem.

Computes log_softmax(tanh(cat(enc, pred)) @ W.T + b) over the vocab dim
for logits of shape [B=4, T=200, U=50, V=1024], fp32.

Strategy (data-parallel over the 800 flattened (b,t) rows, 100 per core):

  Key identity: sum_v exp(e[t,v] + p[u,v]) = exp(e[t,:]) . exp(p[u,:]),
  so the log-softmax denominator for ALL 5000x1024 logits per core is a
  single [100,1024] x [1024,50] matmul over exp'd projections -- no exp
  over the big tensor at all.

  setup (per core, on device):
    teT  = tanh(encT_slice)            [512, 100]  (ACT)
    tpT  = tanh(predT_slice)           [512, 50]   (ACT)
    enc_p  = teT.T @ WeT               [100, 1024] (PE, psum f32)
    pred_b = tpT.T @ WpT + b           [50, 1024]  (PE)
    ee, ep = exp(enc_p), exp(pred_b)   (ACT, bf16)
    eeT, epT = transpose(ee), transpose(ep)        (PE transposes)
    S    = eeT.T-contraction: S[t,u] = ee[t,:].ep[u,:]   (PE, 8 matmuls)
    nlse = -ln(S)                      [100, 50]   (ACT + DVE negate)
    nlse --DMA--> DRAM --DMA--> nlse_rows [128, 40]  (row-major flatten)
    M_A  = [pred_b ; enc_p[0:50]]      [100, 1024] (SBUF-SBUF DMA)
    M_B  = [pred_b ; enc_p[50:100]]    [100, 1024]
  main loop over 40 row-tiles of 128 rows (row r = t*50 + u):
    x    = comb_k.T @ M                (PE, ONE one-hot matmul per 512-
           chunk: comb has the u one-hot in rows 0-49 and the t one-hot
           in rows 50-99, so a single pass does the broadcast-add)
    out  = x + nlse_rows[:,k]          (DVE tensor_scalar / ACT Identity
           with per-partition bias, alternating tiles; bf16 out)
    DMA out tile -> DRAM (sync / gpsimd queues alternating)

  Output is written bf16 (halves the 20.5MB/core store) and upcast to
  fp32 on the host; bf16 rounding is ~0.4% relative, far inside the
  tolerance.
"""

import numpy as np

import concourse.bass as bass
import concourse.bacc as bacc
import concourse.tile as tile
from concourse import mybir
from concourse.bass_utils import run_bass_kernel_spmd

# Problem shapes (hardcoded per contract).
B, T, U, D, V = 4, 200, 50, 512, 1024
N_CORES = 8
BT = B * T                     # 800 flattened (b,t) rows
TPC = BT // N_CORES            # 100 (b,t) rows per core
ROWS = TPC * U                 # 5000 output rows per core
P = 128
NT = (ROWS + P - 1) // P       # 40 row-tiles per core
NV = V // 512                  # psum-bank limit: 512 f32 per matmul out
DC = D // P                    # 4 contraction chunks of 128 for D=512
VC = V // P                    # 8 vocab chunks of 128 (transposes / S)
KSPLIT = 19                    # tiles < 19 have all t < 50; tile 19 spans

f32 = mybir.dt.float32
bf16 = mybir.dt.bfloat16

TRACE = False
LAST_RESULT = None

_CACHE = {}


def _patch_act_tables():
    """Pin Exp/Ln to the one table set containing both, so the activation
    table-load pass never alternates sets mid-kernel."""
    if getattr(bacc, "_joint_act_patch", False):
        return
    orig = bacc.get_activation_tables

    def patched(arch):
        t = dict(orig(arch))
        keep = "natural_log_exp_and_others"
        drop = {mybir.ActivationFunctionType.Exp, mybir.ActivationFunctionType.Ln}
        for name, fns in t.items():
            if name != keep:
                t[name] = set(fns) - drop
        return t

    bacc.get_activation_tables = patched
    bacc._joint_act_patch = True


def _build_indicators():
    """Per-row-tile one-hot stationary operands, shared by all cores.

    comb[k] is [100, 128]: rows 0-49 are the u one-hot (row u has a 1 in
    column c iff u(r)=u for r=128k+c), rows 50-99 the t one-hot relative
    to the M_A/M_B half (t for k<19, t-50 for k>19).  Tile 19 straddles
    the halves and uses separate u19/t19 one-hots the classic way.
    Columns for rows >= ROWS (tail of the last tile) are all-zero.
    """
    comb = np.zeros((NT, TPC, P), dtype=np.float32)
    for k in range(NT):
        if k == KSPLIT:
            continue
        r = np.arange(k * P, min((k + 1) * P, ROWS))
        c = np.arange(len(r))
        off = 50 if k > KSPLIT else 0
        comb[k, (r % U)[c], c] = 1.0
        comb[k, 50 + (r // U)[c] - off, c] = 1.0
    r = np.arange(KSPLIT * P, (KSPLIT + 1) * P)
    c = np.arange(P)
    u19 = np.zeros((U, P), dtype=np.float32)
    t19 = np.zeros((TPC, P), dtype=np.float32)
    u19[r % U, c] = 1.0
    t19[r // U, c] = 1.0
    return np.ascontiguousarray(comb.transpose(1, 0, 2)), u19, t19


def _build_program():
    _patch_act_tables()
    nc = bacc.Bacc("TRN2", target_bir_lowering=False, debug=False,
                   num_devices=N_CORES)

    encT = nc.dram_tensor("encT", [D, TPC], f32, kind="ExternalInput")
    predT = nc.dram_tensor("predT", [D, U], f32, kind="ExternalInput")
    # W in bf16: halves the 4MB load and runs matmuls at full PE rate; the
    # ~1e-3 relative rounding is far inside the output tolerance.
    wT = nc.dram_tensor("wT", [2 * D, V], bf16, kind="ExternalInput")
    bias = nc.dram_tensor("bias", [V], f32, kind="ExternalInput")
    comb = nc.dram_tensor("comb", [TPC, NT, P], bf16, kind="ExternalInput")
    u19 = nc.dram_tensor("u19", [U, P], bf16, kind="ExternalInput")
    t19 = nc.dram_tensor("t19", [TPC, P], bf16, kind="ExternalInput")
    ident = nc.dram_tensor("ident", [P, P], bf16, kind="ExternalInput")
    lse_scr = nc.dram_tensor("lse_scr", [NT * P], f32, kind="Internal")
    out = nc.dram_tensor("out", [ROWS, V], bf16, kind="ExternalOutput")

    Act = mybir.ActivationFunctionType

    with tile.TileContext(nc) as tc:
        with (
            tc.tile_pool(name="consts", bufs=1) as consts,
            tc.tile_pool(name="psum", bufs=4, space=bass.MemorySpace.PSUM) as psum,
            tc.tile_pool(name="outs", bufs=6) as outs,
        ):
            # ---- input DMAs ----
            # sync queue: the setup critical path (encT first), then We/Wp
            encT_sb = consts.tile([P, DC, TPC], f32)
            nc.sync.dma_start(out=encT_sb[:], in_=encT.ap().rearrange(
                "(c p) t -> p c t", p=P))
            predT_sb = consts.tile([P, DC, U], f32)
            nc.sync.dma_start(out=predT_sb[:], in_=predT.ap().rearrange(
                "(c p) u -> p c u", p=P))
            b_sb = consts.tile([1, V], f32)
            nc.sync.dma_start(out=b_sb[:], in_=bias.ap().rearrange(
                "(p v) -> p v", p=1))
            wt_sb = consts.tile([P, 2 * DC, V], bf16)
            wT_r = wT.ap().rearrange("(c p) v -> p c v", p=P)
            for c in range(2 * DC):
                eng = nc.sync if c % 2 == 0 else nc.gpsimd
                eng.dma_start(out=wt_sb[:, c, :], in_=wT_r[:, c, :])
            # gpsimd queue: indicators (first tiles' slice first)
            comb_sb = consts.tile([TPC, NT, P], bf16)
            nc.gpsimd.dma_start(out=comb_sb[:, :6, :], in_=comb.ap()[:, :6, :])
            nc.gpsimd.dma_start(out=comb_sb[:, 6:, :], in_=comb.ap()[:, 6:, :])
            u19_sb = consts.tile([U, P], bf16)
            nc.gpsimd.dma_start(out=u19_sb[:], in_=u19.ap())
            t19_sb = consts.tile([TPC, P], bf16)
            nc.gpsimd.dma_start(out=t19_sb[:], in_=t19.ap())
            ident_sb = consts.tile([P, P], bf16)
            nc.gpsimd.dma_start(out=ident_sb[:], in_=ident.ap())
            ones_u = consts.tile([1, U], f32)
            nc.vector.memset(ones_u[:], 1.0)

            # ---- tanh of activations (d on partitions), bf16 out ----
            teT = consts.tile([P, DC, TPC], bf16)
            nc.scalar.activation(teT[:], encT_sb[:], Act.Tanh)
            tpT = consts.tile([P, DC, U], bf16)
            nc.scalar.activation(tpT[:], predT_sb[:], Act.Tanh)

            # ---- enc_p[t, v] = sum_d teT[d, t] * We[v, d] ----
            enc_ps = psum.tile([P, V], f32, tag="x")
            for vc in range(NV):
                sl = slice(vc * 512, (vc + 1) * 512)
                for c in range(DC):
                    nc.tensor.matmul(enc_ps[:TPC, sl], teT[:, c, :],
                                     wt_sb[:, c, sl],
                                     start=(c == 0), stop=(c == DC - 1))

            # ---- pred_b[u, v] = sum_d tpT[d, u] * Wp[v, d] + b[v] ----
            pred_ps = psum.tile([P, V], f32, tag="x")
            for vc in range(NV):
                sl = slice(vc * 512, (vc + 1) * 512)
                for c in range(DC):
                    nc.tensor.matmul(pred_ps[:U, sl], tpT[:, c, :],
                                     wt_sb[:, DC + c, sl],
                                     start=(c == 0), stop=False)
                nc.tensor.matmul(pred_ps[:U, sl], ones_u[:], b_sb[:, sl],
                                 start=False, stop=True)

            # ---- exp'd projections (for the lse matmul) + bf16 copies ----
            ee = consts.tile([TPC, V], bf16)
            nc.scalar.activation(ee[:], enc_ps[:TPC, :], Act.Exp)
            ep = consts.tile([U, V], bf16)
            nc.scalar.activation(ep[:], pred_ps[:U, :], Act.Exp)
            enc_sb = consts.tile([TPC, V], bf16)
            nc.vector.tensor_copy(enc_sb[:], enc_ps[:TPC, :])
            pred_sb = consts.tile([U, V], bf16)
            nc.vector.tensor_copy(pred_sb[:], pred_ps[:U, :])

            # ---- M_A/M_B: stacked moving operands for the fused one-hot
            #      matmul (partition-crossing copies -> SBUF-SBUF DMA) ----
            m_a = consts.tile([TPC, V], bf16)
            m_b = consts.tile([TPC, V], bf16)
            nc.gpsimd.dma_start(out=m_a[:U, :], in_=pred_sb[:])
            nc.gpsimd.dma_start(out=m_b[:U, :], in_=pred_sb[:])
            nc.gpsimd.dma_start(out=m_a[U:, :], in_=enc_sb[:50, :])
            nc.gpsimd.dma_start(out=m_b[U:, :], in_=enc_sb[50:, :])

            # ---- transposes: eeT[v, t], epT[v, u] (PE, bf16 psum) ----
            eeT_ps = psum.tile([P, VC, TPC], bf16, tag="x")
            eeT_sb = consts.tile([P, VC, TPC], bf16)
            for c in range(VC):
                nc.tensor.transpose(eeT_ps[:, c, :],
                                    ee[:, c * P:(c + 1) * P],
                                    ident_sb[:TPC, :TPC])
                nc.vector.tensor_copy(eeT_sb[:, c, :], eeT_ps[:, c, :])
            epT_ps = psum.tile([P, VC, U], bf16, tag="x")
            epT_sb = consts.tile([P, VC, U], bf16)
            for c in range(VC):
                nc.tensor.transpose(epT_ps[:, c, :],
                                    ep[:, c * P:(c + 1) * P],
                                    ident_sb[:U, :U])
                nc.scalar.activation(epT_sb[:, c, :], epT_ps[:, c, :],
                                     Act.Copy)

            # ---- S[t, u] = ee[t,:] . ep[u,:];  nlse = -ln(S) ----
            s_ps = psum.tile([P, V], f32, tag="x")
            for c in range(VC):
                nc.tensor.matmul(s_ps[:TPC, :U], eeT_sb[:, c, :],
                                 epT_sb[:, c, :],
                                 start=(c == 0), stop=(c == VC - 1))
            lse_sb = consts.tile([TPC, U], f32)
            nc.scalar.activation(lse_sb[:], s_ps[:TPC, :U], Act.Ln)
            nlse = consts.tile([TPC, U], f32)
            nc.vector.tensor_scalar_mul(nlse[:], lse_sb[:], -1.0)

            # ---- flatten nlse[t,u] -> per-row scalars [128, NT] via a
            #      DRAM round trip (row-major (t,u) IS row order) ----
            nc.sync.dma_start(
                out=lse_scr.ap()[:ROWS].rearrange("(t u) -> t u", u=U),
                in_=nlse[:])
            nlse_rows = consts.tile([P, NT], f32)
            nc.sync.dma_start(
                out=nlse_rows[:],
                in_=lse_scr.ap().rearrange("(k p) -> p k", p=P))

            # ---- main loop over row tiles ----
            for k in range(NT):
                r0 = k * P
                rows = min(P, ROWS - r0)
                x_ps = psum.tile([P, V], f32, tag="x")
                if k == KSPLIT:
                    for vc in range(NV):
                        sl = slice(vc * 512, (vc + 1) * 512)
                        nc.tensor.matmul(x_ps[:, sl], u19_sb[:],
                                         pred_sb[:, sl],
                                         start=True, stop=False)
                        nc.tensor.matmul(x_ps[:, sl], t19_sb[:],
                                         enc_sb[:, sl],
                                         start=False, stop=True)
                else:
                    m = m_a if k < KSPLIT else m_b
                    for vc in range(NV):
                        sl = slice(vc * 512, (vc + 1) * 512)
                        nc.tensor.matmul(x_ps[:, sl], comb_sb[:, k, :],
                                         m[:, sl], start=True, stop=True)
                o = outs.tile([P, V], bf16)
                if k % 2 == 0:
                    nc.vector.tensor_scalar_add(o[:rows], x_ps[:rows],
                                                nlse_rows[:rows, k:k + 1])
                else:
                    nc.scalar.activation(o[:rows], x_ps[:rows], Act.Identity,
                                         bias=nlse_rows[:rows, k:k + 1])
                eng = nc.sync if k % 2 == 0 else nc.gpsimd
                eng.dma_start(out=out.ap()[r0:r0 + rows, :], in_=o[:rows])

    nc.compile()
    return nc


def kernel(enc_out, pred_out, W, b):
    global LAST_RESULT
    enc_out = np.asarray(enc_out, dtype=np.float32)
    pred_out = np.asarray(pred_out, dtype=np.float32)
    W = np.asarray(W, dtype=np.float32)
    b = np.asarray(b, dtype=np.float32)

    if "nc" not in _CACHE:
        _CACHE["nc"] = _build_program()
        _CACHE["ind"] = _build_indicators()
    nc = _CACHE["nc"]
    comb, u19, t19 = _CACHE["ind"]

    import ml_dtypes
    wT = np.ascontiguousarray(W.T).astype(ml_dtypes.bfloat16)   # [2D, V]
    enc_flat = enc_out.reshape(BT, D)                 # [800, 512]
    ident = np.eye(P, dtype=ml_dtypes.bfloat16)
    comb_bf = comb.astype(ml_dtypes.bfloat16)
    u19_bf = u19.astype(ml_dtypes.bfloat16)
    t19_bf = t19.astype(ml_dtypes.bfloat16)

    in_maps = []
    for c in range(N_CORES):
        bt0 = c * TPC
        b_idx = bt0 // T
        in_maps.append({
            "encT": np.ascontiguousarray(enc_flat[bt0:bt0 + TPC].T),
            "predT": np.ascontiguousarray(pred_out[b_idx].T),
            "wT": wT,
            "bias": b,
            "comb": comb_bf,
            "u19": u19_bf,
            "t19": t19_bf,
            "ident": ident,
        })

    res = run_bass_kernel_spmd(nc, in_maps, core_ids=list(range(N_CORES)),
                               trace=TRACE)
    LAST_RESULT = res
    full = np.concatenate([r["out"] for r in res.results], axis=0)
    return full.reshape(B, T, U, V).astype(np.float32)


# revision 5
# speedup vs baseline: 1.5106x; 1.0734x over previous
"""Trainium2 Bass kernel for the RNN-T JointNetwork problem.

Computes log_softmax(tanh(cat(enc, pred)) @ W.T + b) over the vocab dim
for logits of shape [B=4, T=200, U=50, V=1024], fp32.

Strategy (data-parallel over the 800 flattened (b,t) rows, 100 per core):

  Key identity: sum_v exp(e[t,v] + p[u,v]) = exp(e[t,:]) . exp(p[u,:]),
  so the log-softmax denominator for ALL 5000x1024 logits per core is a
  single [100,1024] x [1024,50] matmul over exp'd projections -- no exp
  over the big tensor.

  setup (per core, on device):
    teT  = tanh(encT_slice)            [512, 100]  (ACT)
    tpT  = tanh(predT_slice)           [512, 50]   (ACT)
    enc_p  = teT.T @ WeT               [100, 1024] (PE, psum f32)
    pred_b = tpT.T @ WpT + b           [50, 1024]  (PE)
    ee, ep = exp(enc_p), exp(pred_b)   (ACT, bf16)
    eeT, epT = transpose(ee), transpose(ep)        (PE transposes)
    S[t,u] = ee[t,:] . ep[u,:]                     (PE, 8 matmuls)
    nlse = -ln(S)                      [100, 50]   (ACT + DVE negate)
    nlse --DMA--> DRAM --DMA--> nlse_rows [128,40] (row-major flatten)
    M_A  = [pred_b ; enc_p[0:50]]      [100, 1024] (SBUF-SBUF DMA)
    M_B  = [pred_b ; enc_p[50:100]]    [100, 1024]
  main loop over 40 row-tiles of 128 rows (row r = t*50 + u):
    x    = comb_k.T @ M                (PE, ONE one-hot matmul per 512-
           chunk: comb has the u one-hot in rows 0-49 and the t one-hot
           in rows 50-99, so a single pass does the broadcast-add)
    tiles 0..HYB-1 (hybrid, so output DMA starts before the nlse round
    trip lands): lse from ACT exp+accum_out+ln on the tile itself.
    tiles HYB..: out = x + nlse_rows[:,k] (DVE tensor_scalar / ACT
           Identity-with-bias, alternating; bf16 out)
    DMA out per PAIR of tiles -> DRAM (sync / gpsimd alternating)

  Output is written bf16 (halves the 20.5MB/core store) and upcast to
  fp32 on the host; bf16 rounding is ~0.4% relative, far inside the
  tolerance.
"""

import numpy as np

import concourse.bass as bass
import concourse.bacc as bacc
import concourse.tile as tile
from concourse import mybir
from concourse.bass_utils import run_bass_kernel_spmd

# Problem shapes (hardcoded per contract).
B, T, U, D, V = 4, 200, 50, 512, 1024
N_CORES = 8
BT = B * T                     # 800 flattened (b,t) rows
TPC = BT // N_CORES            # 100 (b,t) rows per core
ROWS = TPC * U                 # 5000 output rows per core
P = 128
NT = (ROWS + P - 1) // P       # 40 row-tiles per core
NV = V // 512                  # psum-bank limit: 512 f32 per matmul out
DC = D // P                    # 4 contraction chunks of 128 for D=512
VC = V // P                    # 8 vocab chunks of 128 (transposes / S)
KSPLIT = 19                    # tiles < 19 have all t < 50; tile 19 spans
HYB = 8                        # tiles 0..7 compute lse locally (latency)

f32 = mybir.dt.float32
bf16 = mybir.dt.bfloat16

TRACE = False
LAST_RESULT = None

_CACHE = {}


def _patch_act_tables():
    """Pin Exp/Ln to the one table set containing both, so the activation
    table-load pass never alternates sets mid-kernel."""
    if getattr(bacc, "_joint_act_patch", False):
        return
    orig = bacc.get_activation_tables

    def patched(arch):
        t = dict(orig(arch))
        keep = "natural_log_exp_and_others"
        drop = {mybir.ActivationFunctionType.Exp, mybir.ActivationFunctionType.Ln}
        for name, fns in t.items():
            if name != keep:
                t[name] = set(fns) - drop
        return t

    bacc.get_activation_tables = patched
    bacc._joint_act_patch = True


def _build_indicators():
    """Per-row-tile one-hot stationary operands, shared by all cores.

    comb[k] is [100, 128]: rows 0-49 are the u one-hot (row u has a 1 in
    column c iff u(r)=u for r=128k+c), rows 50-99 the t one-hot relative
    to the M_A/M_B half (t for k<19, t-50 for k>19).  Tile 19 straddles
    the halves and uses separate u19/t19 one-hots the classic way.
    Columns for rows >= ROWS (tail of the last tile) are all-zero.
    """
    comb = np.zeros((NT, TPC, P), dtype=np.float32)
    for k in range(NT):
        if k == KSPLIT:
            continue
        r = np.arange(k * P, min((k + 1) * P, ROWS))
        c = np.arange(len(r))
        off = 50 if k > KSPLIT else 0
        comb[k, (r % U)[c], c] = 1.0
        comb[k, 50 + (r // U)[c] - off, c] = 1.0
    r = np.arange(KSPLIT * P, (KSPLIT + 1) * P)
    c = np.arange(P)
    u19 = np.zeros((U, P), dtype=np.float32)
    t19 = np.zeros((TPC, P), dtype=np.float32)
    u19[r % U, c] = 1.0
    t19[r // U, c] = 1.0
    return np.ascontiguousarray(comb.transpose(1, 0, 2)), u19, t19


def _build_program():
    _patch_act_tables()
    nc = bacc.Bacc("TRN2", target_bir_lowering=False, debug=False,
                   num_devices=N_CORES)

    encT = nc.dram_tensor("encT", [D, TPC], f32, kind="ExternalInput")
    predT = nc.dram_tensor("predT", [D, U], f32, kind="ExternalInput")
    # W in bf16: halves the 4MB load and runs matmuls at full PE rate; the
    # ~1e-3 relative rounding is far inside the output tolerance.
    wT = nc.dram_tensor("wT", [2 * D, V], bf16, kind="ExternalInput")
    bias = nc.dram_tensor("bias", [V], f32, kind="ExternalInput")
    comb = nc.dram_tensor("comb", [TPC, NT, P], bf16, kind="ExternalInput")
    u19 = nc.dram_tensor("u19", [U, P], bf16, kind="ExternalInput")
    t19 = nc.dram_tensor("t19", [TPC, P], bf16, kind="ExternalInput")
    ident = nc.dram_tensor("ident", [P, P], bf16, kind="ExternalInput")
    lse_scr = nc.dram_tensor("lse_scr", [NT * P], f32, kind="Internal")
    out = nc.dram_tensor("out", [ROWS, V], bf16, kind="ExternalOutput")

    Act = mybir.ActivationFunctionType

    with tile.TileContext(nc) as tc:
        with (
            tc.tile_pool(name="consts", bufs=1) as consts,
            tc.tile_pool(name="psum", bufs=4, space=bass.MemorySpace.PSUM) as psum,
            tc.tile_pool(name="outs", bufs=4) as outs,
            tc.tile_pool(name="scratch", bufs=2) as scratch,
            tc.tile_pool(name="small", bufs=8) as small,
        ):
            # ---- input DMAs, spread over the three issue queues so the
            #      setup matmuls aren't paced by issue serialization ----
            # sync: encT first (gates tanh), then We chunks 0-1
            encT_sb = consts.tile([P, DC, TPC], f32)
            nc.sync.dma_start(out=encT_sb[:], in_=encT.ap().rearrange(
                "(c p) t -> p c t", p=P))
            wt_sb = consts.tile([P, 2 * DC, V], bf16)
            wT_r = wT.ap().rearrange("(c p) v -> p c v", p=P)
            nc.sync.dma_start(out=wt_sb[:, 0, :], in_=wT_r[:, 0, :])
            nc.sync.dma_start(out=wt_sb[:, 1, :], in_=wT_r[:, 1, :])
            # scalar: We chunks 2-3 (issued before tanh needs the engine)
            nc.scalar.dma_start(out=wt_sb[:, 2, :], in_=wT_r[:, 2, :])
            nc.scalar.dma_start(out=wt_sb[:, 3, :], in_=wT_r[:, 3, :])
            # gpsimd: predT, Wp chunks, bias, ident, indicators
            predT_sb = consts.tile([P, DC, U], f32)
            nc.gpsimd.dma_start(out=predT_sb[:], in_=predT.ap().rearrange(
                "(c p) u -> p c u", p=P))
            for c in range(DC, 2 * DC):
                nc.gpsimd.dma_start(out=wt_sb[:, c, :], in_=wT_r[:, c, :])
            b_sb = consts.tile([1, V], f32)
            nc.gpsimd.dma_start(out=b_sb[:], in_=bias.ap().rearrange(
                "(p v) -> p v", p=1))
            ident_sb = consts.tile([P, P], bf16)
            nc.gpsimd.dma_start(out=ident_sb[:], in_=ident.ap())
            comb_sb = consts.tile([TPC, NT, P], bf16)
            nc.gpsimd.dma_start(out=comb_sb[:, :6, :], in_=comb.ap()[:, :6, :])
            u19_sb = consts.tile([U, P], bf16)
            nc.gpsimd.dma_start(out=u19_sb[:], in_=u19.ap())
            t19_sb = consts.tile([TPC, P], bf16)
            nc.gpsimd.dma_start(out=t19_sb[:], in_=t19.ap())
            nc.gpsimd.dma_start(out=comb_sb[:, 6:, :], in_=comb.ap()[:, 6:, :])
            ones_u = consts.tile([1, U], f32)
            nc.vector.memset(ones_u[:], 1.0)

            # ---- tanh of activations (d on partitions), bf16 out ----
            teT = consts.tile([P, DC, TPC], bf16)
            nc.scalar.activation(teT[:], encT_sb[:], Act.Tanh)
            tpT = consts.tile([P, DC, U], bf16)
            nc.scalar.activation(tpT[:], predT_sb[:], Act.Tanh)

            # ---- enc_p[t, v] = sum_d teT[d, t] * We[v, d] ----
            enc_ps = psum.tile([P, V], f32, tag="x")
            for vc in range(NV):
                sl = slice(vc * 512, (vc + 1) * 512)
                for c in range(DC):
                    nc.tensor.matmul(enc_ps[:TPC, sl], teT[:, c, :],
                                     wt_sb[:, c, sl],
                                     start=(c == 0), stop=(c == DC - 1))

            # ---- pred_b[u, v] = sum_d tpT[d, u] * Wp[v, d] + b[v] ----
            pred_ps = psum.tile([P, V], f32, tag="x")
            for vc in range(NV):
                sl = slice(vc * 512, (vc + 1) * 512)
                for c in range(DC):
                    nc.tensor.matmul(pred_ps[:U, sl], tpT[:, c, :],
                                     wt_sb[:, DC + c, sl],
                                     start=(c == 0), stop=False)
                nc.tensor.matmul(pred_ps[:U, sl], ones_u[:], b_sb[:, sl],
                                 start=False, stop=True)

            # ---- exp'd projections (for the lse matmul) + bf16 copies ----
            ee = consts.tile([TPC, V], bf16)
            nc.scalar.activation(ee[:], enc_ps[:TPC, :], Act.Exp)
            ep = consts.tile([U, V], bf16)
            nc.scalar.activation(ep[:], pred_ps[:U, :], Act.Exp)
            enc_sb = consts.tile([TPC, V], bf16)
            nc.vector.tensor_copy(enc_sb[:], enc_ps[:TPC, :])
            pred_sb = consts.tile([U, V], bf16)
            nc.scalar.activation(pred_sb[:], pred_ps[:U, :], Act.Copy)

            # ---- M_A/M_B: stacked moving operands for the fused one-hot
            #      matmul (partition-crossing copies -> SBUF-SBUF DMA) ----
            m_a = consts.tile([TPC, V], bf16)
            m_b = consts.tile([TPC, V], bf16)
            nc.gpsimd.dma_start(out=m_a[:U, :], in_=pred_sb[:])
            nc.gpsimd.dma_start(out=m_a[U:, :], in_=enc_sb[:50, :])
            nc.gpsimd.dma_start(out=m_b[:U, :], in_=pred_sb[:])
            nc.gpsimd.dma_start(out=m_b[U:, :], in_=enc_sb[50:, :])

            # ---- transposes: eeT[v, t], epT[v, u] (PE, bf16 psum) ----
            epT_ps = psum.tile([P, VC, U], bf16, tag="x")
            epT_sb = consts.tile([P, VC, U], bf16)
            for c in range(VC):
                nc.tensor.transpose(epT_ps[:, c, :],
                                    ep[:, c * P:(c + 1) * P],
                                    ident_sb[:U, :U])
                nc.scalar.activation(epT_sb[:, c, :], epT_ps[:, c, :],
                                     Act.Copy)
            eeT_ps = psum.tile([P, VC, TPC], bf16, tag="x")
            eeT_sb = consts.tile([P, VC, TPC], bf16)
            for c in range(VC):
                nc.tensor.transpose(eeT_ps[:, c, :],
                                    ee[:, c * P:(c + 1) * P],
                                    ident_sb[:TPC, :TPC])
                nc.vector.tensor_copy(eeT_sb[:, c, :], eeT_ps[:, c, :])

            # ---- S[t, u] = ee[t,:] . ep[u,:];  nlse = -ln(S) ----
            s_ps = psum.tile([P, V], f32, tag="x")
            for c in range(VC):
                nc.tensor.matmul(s_ps[:TPC, :U], eeT_sb[:, c, :],
                                 epT_sb[:, c, :],
                                 start=(c == 0), stop=(c == VC - 1))
            lse_sb = consts.tile([TPC, U], f32)
            nc.scalar.activation(lse_sb[:], s_ps[:TPC, :U], Act.Ln)
            nlse = consts.tile([TPC, U], f32)
            nc.vector.tensor_scalar_mul(nlse[:], lse_sb[:], -1.0)

            # ---- flatten nlse[t,u] -> per-row scalars [128, NT] via a
            #      DRAM round trip (row-major (t,u) IS row order) ----
            nc.sync.dma_start(
                out=lse_scr.ap()[:ROWS].rearrange("(t u) -> t u", u=U),
                in_=nlse[:])
            nlse_rows = consts.tile([P, NT], f32)
            nc.gpsimd.dma_start(
                out=nlse_rows[:],
                in_=lse_scr.ap().rearrange("(k p) -> p k", p=P))

            # ---- main loop over row tiles, output DMA per pair ----
            o2 = None
            for k in range(NT):
                r0 = k * P
                rows = min(P, ROWS - r0)
                x_ps = psum.tile([P, V], f32, tag="x")
                if k == KSPLIT:
                    for vc in range(NV):
                        sl = slice(vc * 512, (vc + 1) * 512)
                        nc.tensor.matmul(x_ps[:, sl], u19_sb[:],
                                         pred_sb[:, sl],
                                         start=True, stop=False)
                        nc.tensor.matmul(x_ps[:, sl], t19_sb[:],
                                         enc_sb[:, sl],
                                         start=False, stop=True)
                else:
                    m = m_a if k < KSPLIT else m_b
                    for vc in range(NV):
                        sl = slice(vc * 512, (vc + 1) * 512)
                        nc.tensor.matmul(x_ps[:, sl], comb_sb[:, k, :],
                                         m[:, sl], start=True, stop=True)
                if k % 2 == 0:
                    o2 = outs.tile([P, 2, V], bf16)
                oh = o2[:, k % 2, :]
                if k < HYB:
                    # local lse: ACT exp+accum -> ln; DVE subtract
                    escr = scratch.tile([P, V], f32)
                    sums = small.tile([P, 1], f32)
                    nc.scalar.activation(escr[:], x_ps[:], Act.Exp,
                                         accum_out=sums[:])
                    lse_h = small.tile([P, 1], f32)
                    nc.scalar.activation(lse_h[:], sums[:], Act.Ln)
                    nc.vector.tensor_scalar_sub(oh[:rows], x_ps[:rows],
                                                lse_h[:rows])
                elif k % 2 == 0:
                    nc.vector.tensor_scalar_add(oh[:rows], x_ps[:rows],
                                                nlse_rows[:rows, k:k + 1])
                else:
                    nc.scalar.activation(oh[:rows], x_ps[:rows], Act.Identity,
                                         bias=nlse_rows[:rows, k:k + 1])
                # DMA per pair; the straddling tail pair goes per-tile
                eng = nc.sync if (k // 2) % 2 == 0 else nc.gpsimd
                if k % 2 == 1 and rows == P:
                    pr = (k - 1) * P
                    eng.dma_start(
                        out=out.ap()[pr:pr + 2 * P, :].rearrange(
                            "(two p) v -> p two v", p=P),
                        in_=o2[:])
                elif k % 2 == 1 or rows < P:
                    eng.dma_start(out=out.ap()[r0:r0 + rows, :],
                                  in_=oh[:rows])
                    if k % 2 == 1:
                        eng.dma_start(out=out.ap()[r0 - P:r0, :],
                                      in_=o2[:, 0, :])

    nc.compile()
    return nc


def kernel(enc_out, pred_out, W, b):
    global LAST_RESULT
    enc_out = np.asarray(enc_out, dtype=np.float32)
    pred_out = np.asarray(pred_out, dtype=np.float32)
    W = np.asarray(W, dtype=np.float32)
    b = np.asarray(b, dtype=np.float32)

    if "nc" not in _CACHE:
        _CACHE["nc"] = _build_program()
        _CACHE["ind"] = _build_indicators()
    nc = _CACHE["nc"]
    comb, u19, t19 = _CACHE["ind"]

    import ml_dtypes
    wT = np.ascontiguousarray(W.T).astype(ml_dtypes.bfloat16)   # [2D, V]
    enc_flat = enc_out.reshape(BT, D)                 # [800, 512]
    ident = np.eye(P, dtype=ml_dtypes.bfloat16)
    comb_bf = comb.astype(ml_dtypes.bfloat16)
    u19_bf = u19.astype(ml_dtypes.bfloat16)
    t19_bf = t19.astype(ml_dtypes.bfloat16)

    in_maps = []
    for c in range(N_CORES):
        bt0 = c * TPC
        b_idx = bt0 // T
        in_maps.append({
            "encT": np.ascontiguousarray(enc_flat[bt0:bt0 + TPC].T),
            "predT": np.ascontiguousarray(pred_out[b_idx].T),
            "wT": wT,
            "bias": b,
            "comb": comb_bf,
            "u19": u19_bf,
            "t19": t19_bf,
            "ident": ident,
        })

    res = run_bass_kernel_spmd(nc, in_maps, core_ids=list(range(N_CORES)),
                               trace=TRACE)
    LAST_RESULT = res
    full = np.concatenate([r["out"] for r in res.results], axis=0)
    return full.reshape(B, T, U, V).astype(np.float32)
